# revision 39
# baseline (speedup 1.0000x reference)
"""Trainium2 Bass kernel for BodyConvClothGraphConvolution.

Reference computation (R = C = 8192, D = H = 256):
    X0  = notes @ w                     # (R+C, H)
    top = X0[:R] + weight @ X0[R:]      # (R, H)
    out = concat([relu(top + b), relu(b)*ones(C,H), X0[R:]], axis=0)

Key reassociation: weight @ (Nb @ w) == (weight @ Nb) @ w. Computing
Y = weight_shard @ Nb first makes the big matmul's stationary operand the
raw body notes straight from HBM -- the (R+C,*)-wide projection X0 = notes@w
never has to be materialized per core, eliminating the replicated
notes_body @ w phase (~33us of LDW-stalled PE time per core) entirely.

Sharding (8 cores, zero cross-core communication):
  - weight rows and cloth note rows sharded 8-way (1024 rows/core).
  - body notes replicated (needed whole as the contraction operand).
  - each core computes only its OWN 1024-row slice of X0[R:] for the output.

Per-core kernel (all matmuls bf16 inputs, fp32 PSUM accumulation):
  stream: YT[d,m] += Nb-block-stationary @ W.T-slab      (256 MM, N=512,
          4 persistent PSUM banks, weight streamed 1MB slabs, body-note
          chunks interleaved into the same DMA stream)
  head:   X0bT_own[h,c] = w.T-stationary @ Nb_own.T      (8 MM, N=512,
          slotted into the stream after slab 1 so DMA, not PE, covers it)
  tail:   topT[h,m] = b + w.T @ [NcT | YT]               (16 MM, N=512)
          fp32 bias + relu fused into ACT/DVE copies out of PSUM,
          bf16 stores upcast on the host.
"""

import numpy as np
import ml_dtypes

R, C, D, H = 8192, 8192, 256, 256
NCORES = 8
MSHARD = R // NCORES          # 1024 output rows (and own body rows) per core
NCT = C // 128                # 64 body-vertex 128-blocks (contraction dim)
NDT = D // 128                # 2 d-tiles
NHT = H // 128                # 2 h-tiles
NCB = NCT // 4                # 16 weight DMA slabs (4 c-blocks each)

BF16 = ml_dtypes.bfloat16

_CACHE = {}


def _build_nc(reps=1, loop_iters=1):
    """Build + compile the SPMD Bass program (same program for all cores)."""
    import concourse.bass as bass
    import concourse.bacc as bacc
    import concourse.tile as tile
    from concourse import mybir

    fp32 = mybir.dt.float32
    bf16 = mybir.dt.bfloat16

    nc = bacc.Bacc("TRN2", target_bir_lowering=False, debug=False,
                   num_devices=NCORES)

    # DRAM I/O (per-core shapes)
    wt_d = nc.dram_tensor("wt", [128, NDT * H], bf16,
                          kind="ExternalInput").ap()
    b2_d = nc.dram_tensor("b2", [128, NHT], fp32, kind="ExternalInput").ap()
    nbp_d = nc.dram_tensor("nbp", [128, NCT * D], bf16,
                           kind="ExternalInput").ap()
    nbto_d = nc.dram_tensor("nbto", [128, NDT * MSHARD], bf16,
                            kind="ExternalInput").ap()
    nct_d = nc.dram_tensor("nct", [128, NDT * MSHARD], bf16,
                           kind="ExternalInput").ap()
    wpe_d = nc.dram_tensor("wpe", [NCB, 128, 4 * MSHARD], bf16,
                           kind="ExternalInput").ap()
    top_d = nc.dram_tensor("topt_out", [NHT, 128, MSHARD], bf16,
                           kind="ExternalOutput").ap()
    x0b_d = nc.dram_tensor("x0bt_out", [NHT, 128, MSHARD], bf16,
                           kind="ExternalOutput").ap()

    def body(tc, const_pool, wpe_pool, psA_pool, psY_pool, out_pool):
        wt_sb = const_pool.tile([128, NDT * H], bf16)
        b2_sb = const_pool.tile([128, NHT], fp32)
        nbto_sb = const_pool.tile([128, NDT * MSHARD], bf16)
        nb_sb = const_pool.tile([128, NCT * D], bf16)
        nct_sb = const_pool.tile([128, NDT * MSHARD], bf16)
        y_sb = const_pool.tile([128, NDT * MSHARD], bf16)
        xo_sb = const_pool.tile([128, NHT * MSHARD], bf16)

        # minimal-prefix DMA order: descriptor generation is ~0.65us per DMA
        # on Sync, so the first matmul's operands must sit behind as few
        # descs (and bytes) as possible
        nc.sync.dma_start(out=nb_sb[:, :D], in_=nbp_d[:, :D])

        # ---- stream: YT[d, m] = sum_c Nb[c,d] * weight[m,c] ----
        yps = [psY_pool.tile([128, 512], fp32, name=f"yps{g}", tag=f"yps{g}")
               for g in range(NDT * 2)]
        for cb in range(NCB):
            wslab = wpe_pool.tile([128, 4 * MSHARD], bf16)
            if cb == 0:
                # fine-grained first slab so j=0 unblocks early
                nc.sync.dma_start(out=wslab[:, :512], in_=wpe_d[0, :, :512])
                nc.sync.dma_start(out=wslab[:, 512:MSHARD],
                                  in_=wpe_d[0, :, 512:MSHARD])
                nc.sync.dma_start(out=nb_sb[:, D:4 * D],
                                  in_=nbp_d[:, D:4 * D])
                nc.sync.dma_start(out=wslab[:, MSHARD:],
                                  in_=wpe_d[0, :, MSHARD:])
            else:
                nc.sync.dma_start(out=wslab[:, :], in_=wpe_d[cb])
            if cb + 1 < NCB:
                # body-note chunk for the NEXT slab rides the same stream
                lo, hi = 4 * (cb + 1) * D, 4 * (cb + 2) * D
                nc.sync.dma_start(out=nb_sb[:, lo:hi], in_=nbp_d[:, lo:hi])
            if cb == 1:
                # head/tail-only operands: desc-gen after slab 1 so the
                # stream's second slab isn't delayed behind them
                nc.sync.dma_start(out=wt_sb[:, :], in_=wt_d[:, :])
                nc.sync.dma_start(out=nbto_sb[:, :], in_=nbto_d[:, :])
                nc.sync.dma_start(out=b2_sb[:, :], in_=b2_d[:, :])
            if cb == NCB - 2:
                # cloth notes are a tail-only operand
                nc.sync.dma_start(out=nct_sb[:, :], in_=nct_d[:, :])
            if cb == NCB - 1:
                # own-X0b store (bf16) rides the end-of-stream DMA window
                for ht in range(NHT):
                    nc.sync.dma_start(
                        out=x0b_d[ht],
                        in_=xo_sb[:, ht * MSHARD:(ht + 1) * MSHARD])
            for j in range(4):
                ct = cb * 4 + j
                # for the very first c-block, follow DMA arrival order
                # (cols 0:512 land first); elsewhere keep dt-major order
                loop = ([(dt, mc) for mc in range(2) for dt in range(NDT)]
                        if ct == 0 else
                        [(dt, mc) for dt in range(NDT) for mc in range(2)])
                for dt, mc in loop:
                    nc.tensor.matmul(
                        yps[dt * 2 + mc][:, :],
                        lhsT=nb_sb[:, ct * D + dt * 128:
                                   ct * D + (dt + 1) * 128],
                        rhs=wslab[:, j * MSHARD + mc * 512:
                                  j * MSHARD + (mc + 1) * 512],
                        start=(ct == 0), stop=(ct == NCT - 1),
                    )
            if cb == 1:
                # head: X0bT_own[h, c] = sum_d w[d,h] * Nb_own[c,d]
                # slotted mid-stream where the PE chain, not DMA, is the
                # limiter; nbto arrived during slab 1
                for ht in range(NHT):
                    for dt in range(NDT):
                        for cc in range(2):
                            ps = psA_pool.tile([128, 512], fp32,
                                               name=f"psa{ht}{cc}",
                                               tag=f"psa{ht}{cc}")
                            nc.tensor.matmul(
                                ps[:, :],
                                lhsT=wt_sb[:, dt * H + ht * 128:
                                           dt * H + (ht + 1) * 128],
                                rhs=nbto_sb[:, dt * MSHARD + cc * 512:
                                            dt * MSHARD + (cc + 1) * 512],
                                start=(dt == 0), stop=(dt == NDT - 1),
                            )
                            if dt == NDT - 1:
                                nc.scalar.copy(
                                    out=xo_sb[:, ht * MSHARD + cc * 512:
                                              ht * MSHARD + (cc + 1) * 512],
                                    in_=ps[:, :])


        # ---- tail: topT[h, m] = b + sum_d w[d,h]*(Nc[m,d] + Y[m,d]) ----
        for dt in range(NDT):
            for mc in range(2):
                nc.vector.tensor_copy(
                    out=y_sb[:, dt * MSHARD + mc * 512:
                             dt * MSHARD + (mc + 1) * 512],
                    in_=yps[dt * 2 + mc][:, :])
        for ht in range(NHT):
            tps = [psA_pool.tile([128, 512], fp32, name=f"tps{ht}{mc}",
                                 tag=f"psa{ht}{mc}") for mc in range(2)]
            for dt in range(NDT):
                for mc in range(2):
                    for si, src in enumerate((nct_sb, y_sb)):
                        nc.tensor.matmul(
                            tps[mc][:, :],
                            lhsT=wt_sb[:, dt * H + ht * 128:
                                       dt * H + (ht + 1) * 128],
                            rhs=src[:, dt * MSHARD + mc * 512:
                                    dt * MSHARD + (mc + 1) * 512],
                            start=(dt == 0 and si == 0),
                            stop=(dt == NDT - 1 and si == 1),
                        )
            for mc in range(2):
                o = out_pool.tile([128, 512], bf16, tag="topout")
                if mc == 0:
                    # relu(x + b): ACT and DVE split the final banks so the
                    # two relu+bias pairs run concurrently
                    nc.scalar.activation(o[:, :], tps[mc][:, :],
                                         mybir.ActivationFunctionType.Relu,
                                         bias=b2_sb[:, ht:ht + 1])
                else:
                    nc.vector.tensor_scalar(
                        out=o[:, :], in0=tps[mc][:, :],
                        scalar1=b2_sb[:, ht:ht + 1], scalar2=0.0,
                        op0=mybir.AluOpType.add, op1=mybir.AluOpType.max)
                # store desc-gen split across engine queues: Sync serializes
                # at ~0.6us per DMA which would delay the final stores
                eng = nc.sync if mc == 0 else nc.gpsimd
                eng.dma_start(out=top_d[ht, :, mc * 512:(mc + 1) * 512],
                              in_=o[:, :])

    with tile.TileContext(nc) as tc:
        with (
            tc.tile_pool(name="const", bufs=1) as const_pool,
            tc.tile_pool(name="wpe", bufs=5) as wpe_pool,
            tc.tile_pool(name="psA", bufs=1, space="PSUM") as psA_pool,
            tc.tile_pool(name="psY", bufs=1, space="PSUM") as psY_pool,
            tc.tile_pool(name="outs", bufs=4) as out_pool,
        ):
            pools = (const_pool, wpe_pool, psA_pool, psY_pool, out_pool)
            if loop_iters > 1:
                with tc.For_i(0, loop_iters, 1,
                              hint_engines=(mybir.EngineType.PE,)):
                    body(tc, *pools)
            else:
                for _rep in range(reps):
                    body(tc, *pools)

    nc.compile()
    return nc


def _get_nc(reps=1, loop_iters=1):
    key = ("nc", reps, loop_iters)
    if key not in _CACHE:
        _CACHE[key] = _build_nc(reps, loop_iters)
    return _CACHE[key]


def _pack_inputs(notes, weight, w, b):
    """Host-side shard + transpose + bf16 cast into per-core in_maps."""
    nb = np.ascontiguousarray(notes[R:]).astype(BF16)       # (C, D)
    ncl = np.ascontiguousarray(notes[:R]).astype(BF16)      # (R, D)
    wq = w.astype(BF16)                                     # (D, H)

    wt = np.ascontiguousarray(
        wq.reshape(NDT, 128, H).transpose(1, 0, 2).reshape(128, NDT * H))
    b2 = np.ascontiguousarray(b.reshape(NHT, 128).T)        # (128, NHT) f32
    # nbp[p, ct*D + d] = Nb[ct*128 + p, d]  (replicated across cores)
    nbp = np.ascontiguousarray(
        nb.reshape(NCT, 128, D).transpose(1, 0, 2).reshape(128, NCT * D))

    in_maps = []
    for k in range(NCORES):
        nbk = nb[k * MSHARD:(k + 1) * MSHARD]               # (MSHARD, D)
        nbto = np.ascontiguousarray(
            nbk.T.reshape(NDT, 128, MSHARD).transpose(1, 0, 2)
            .reshape(128, NDT * MSHARD))
        nck = ncl[k * MSHARD:(k + 1) * MSHARD]              # (MSHARD, D)
        nct = np.ascontiguousarray(
            nck.T.reshape(NDT, 128, MSHARD).transpose(1, 0, 2)
            .reshape(128, NDT * MSHARD))
        wk = weight[k * MSHARD:(k + 1) * MSHARD].astype(BF16)   # (MSHARD, C)
        # wpe[cb, p, j*MSHARD + m] = weight[k*MSHARD+m, (4cb+j)*128 + p]
        wpe = np.ascontiguousarray(
            wk.reshape(MSHARD, NCB, 4, 128).transpose(1, 3, 2, 0)
            .reshape(NCB, 128, 4 * MSHARD))

        in_maps.append({
            "wt": wt, "b2": b2, "nbp": nbp, "nbto": nbto, "nct": nct,
            "wpe": wpe,
        })
    return in_maps


def kernel(notes, weight, w, b):
    from concourse.bass_utils import run_bass_kernel_spmd

    notes = np.asarray(notes, dtype=np.float32)
    weight = np.asarray(weight, dtype=np.float32)
    w = np.asarray(w, dtype=np.float32)
    b = np.asarray(b, dtype=np.float32)

    nc = _get_nc()
    in_maps = _pack_inputs(notes, weight, w, b)
    res = run_bass_kernel_spmd(nc, in_maps, core_ids=list(range(NCORES)),
                               trace=False)

    out = np.empty((R + 2 * C, H), dtype=np.float32)
    for k in range(NCORES):
        r = res.results[k]
        out[k * MSHARD:(k + 1) * MSHARD] = \
            r["topt_out"].reshape(H, MSHARD).T.astype(np.float32)
        out[R + C + k * MSHARD:R + C + (k + 1) * MSHARD] = \
            r["x0bt_out"].reshape(H, MSHARD).T.astype(np.float32)
    out[R:R + C] = np.maximum(b, 0.0)[None, :]
    return out
